# revision 12
# baseline (speedup 1.0000x reference)
"""Trainium2 Bass kernel for the DAN classifier (gather + segment-mean + MLP).

Full computation:
    gathered = embeddings[docs]                    # [B, L, D]
    avg = gathered.sum(1) / doc_lens[:, None]      # [B, D]
    out = relu(relu(avg @ W1 + b1) @ W2 + b2) @ W3 + b3   # [B, C]

The fused gather + segment-sum is a sparse matmul
    doc_sums[b, :] = sum_v CNT[v, b] * embeddings[v, :]
where CNT[v, b] counts occurrences of vocab v in doc b (host-built with one
bincount).  The table is vocab-sharded: core k streams its 12,500 rows
sequentially at full DMA bandwidth, multiplies on the PE against its count
shard, and one fp16 ReduceScatter(add) hands each core the finished sums for
its 32 docs.

Stream phase runs entirely in fp8e4 (e4m3, max 240) with MatmulPerfMode
DoubleRow: each matmul contracts K=256 vocab rows at 0.5 cycles/row.  The
fp32 table is split hi = fp8(x), lo = fp8((x - hi) * 16) and accumulated in
separate PSUM banks, recombined as hi + lo/16 => ~2^-8 relative error.
Counts are small ints (<= 16), exact in fp8.  All tensors are packed
partition-major on the host so every stream DMA moves 5-6 KB contiguous per
partition.  The 1/len mean is folded into a tiny fp16 matmul against
diag(1/len) that also transposes doc-major sums into feature-major MLP
input; the 3-layer MLP runs in fp16 (weights fp16, fp32 PSUM accum).
"""

import numpy as np

# Problem shapes (hardcoded per contract).
V, D = 100000, 300
B, L = 256, 1000
H, C = 512, 5
NCORES = 8
BC = B // NCORES            # docs per core = 32
VSH = V // NCORES           # vocab rows per core = 12500
NCHK = 50                   # 256-row DoubleRow chunks per core (12800 rows)
VSHP = NCHK * 256           # padded shard rows = 12800
TCPG = 5                    # thl chunks per DMA group (6 KB/partition)
NTG = NCHK // TCPG          # 10 thl DMA groups
CCPG = 10                   # cnt chunks per DMA group (5 KB/partition)
NCG = NCHK // CCPG          # 5 cnt DMA groups
LOSCALE = 16.0              # lo-half scale (2^4)

_CACHE = {}


def _build_nc():
    import concourse.bass as bass
    import concourse.bacc as bacc
    import concourse.mybir as mybir
    import concourse.tile as tile

    dt = mybir.dt
    f32 = dt.float32
    fp16 = dt.float16
    fp8 = dt.float8e4
    DR = mybir.MatmulPerfMode.DoubleRow

    nc = bacc.Bacc("TRN2", target_bir_lowering=False, debug=False, num_devices=NCORES)

    # Stream inputs, partition-major: [group][partition][chunk-in-group]...
    thl_d = nc.dram_tensor("thl", [NTG, 128, TCPG, 2, 2 * D], fp8, kind="ExternalInput")
    cnt_d = nc.dram_tensor("cnt", [NCG, 128, CCPG, 2, B], fp8, kind="ExternalInput")
    # MLP weights (fp16) and biases (f32), pre-tiled on host.
    w1_d = nc.dram_tensor("w1", [100, 12 * 128], fp16, kind="ExternalInput")
    w2_d = nc.dram_tensor("w2", [128, 16 * 128], fp16, kind="ExternalInput")
    w3_d = nc.dram_tensor("w3", [128, 4 * C], fp16, kind="ExternalInput")
    b1_d = nc.dram_tensor("b1", [128, 4], f32, kind="ExternalInput")
    b2_d = nc.dram_tensor("b2", [128, 4], f32, kind="ExternalInput")
    b3_d = nc.dram_tensor("b3", [1, C], fp16, kind="ExternalInput")
    ones_d = nc.dram_tensor("ones", [1, BC], fp16, kind="ExternalInput")
    dinv_d = nc.dram_tensor("dinv", [BC, BC], fp16, kind="ExternalInput")
    out_d = nc.dram_tensor("out", [BC, C], f32, kind="ExternalOutput")

    cc_in = nc.dram_tensor("cc_in", [B, D], fp16)
    cc_out = nc.dram_tensor("cc_out", [BC, D], fp16)
    warm_in = nc.dram_tensor("warm_in", [1, 64], fp16)
    warm_out = nc.dram_tensor("warm_out", [NCORES, 64], fp16)

    relu = mybir.ActivationFunctionType.Relu

    with tile.TileContext(nc) as tc:
        with (
            tc.tile_pool(name="const", bufs=1) as cp,
            tc.tile_pool(name="tstream", bufs=3) as tp,
            tc.tile_pool(name="work", bufs=1) as wp,
            tc.tile_pool(name="psacc", bufs=1, space="PSUM") as pp,
            tc.tile_pool(name="psmlp", bufs=4, space="PSUM") as pp2,
        ):
            # Resident count shard: 25.6 KB/partition, streamed in 5 groups on
            # the Act queue while the thl stream runs on the SP queue.  One
            # tile per group so the first matmuls only wait on group 0.
            cnt_sb = [
                cp.tile([128, CCPG, 2, B], fp8, name=f"cnt_sb{g}")
                for g in range(NCG)
            ]
            for g in range(NCG):
                nc.scalar.dma_start(out=cnt_sb[g][:], in_=cnt_d[g])

            # Warm up the collectives stream early so its one-time setup cost
            # is paid under the compute phase, not in front of the real RS.
            nc.gpsimd.collective_compute(
                "AllGather",
                mybir.AluOpType.bypass,
                replica_groups=[list(range(NCORES))],
                ins=[warm_in[:]],
                outs=[warm_out[:]],
            )

            # Tail-only constants ride the Act queue after the counts.
            w1_sb = cp.tile([100, 12 * 128], fp16)
            nc.scalar.dma_start(out=w1_sb[:], in_=w1_d[:])
            w2_sb = cp.tile([128, 16 * 128], fp16)
            nc.scalar.dma_start(out=w2_sb[:], in_=w2_d[:])
            w3_sb = cp.tile([128, 4 * C], fp16)
            nc.scalar.dma_start(out=w3_sb[:], in_=w3_d[:])
            b1_sb = cp.tile([128, 4], f32)
            nc.scalar.dma_start(out=b1_sb[:], in_=b1_d[:])
            b2_sb = cp.tile([128, 4], f32)
            nc.scalar.dma_start(out=b2_sb[:], in_=b2_d[:])
            b3_sb = cp.tile([1, C], fp16)
            nc.scalar.dma_start(out=b3_sb[:], in_=b3_d[:])
            ones_sb = cp.tile([1, BC], fp16)
            nc.scalar.dma_start(out=ones_sb[:], in_=ones_d[:])
            dinv_sb = cp.tile([BC, BC], fp16)
            nc.scalar.dma_start(out=dinv_sb[:], in_=dinv_d[:])

            # Partial doc sums: hi/lo x docs 0:128 / 128:256.
            psA = pp.tile([128, D], f32, tag="psA")
            psB = pp.tile([128, D], f32, tag="psB")
            psLA = pp.tile([128, D], f32, tag="psLA")
            psLB = pp.tile([128, D], f32, tag="psLB")

            for g in range(NTG):
                tt = tp.tile([128, TCPG, 2, 2 * D], fp8)
                nc.sync.dma_start(out=tt[:], in_=thl_d[g])
                for c in range(TCPG):
                    chunk = g * TCPG + c
                    st, sp_ = chunk == 0, chunk == NCHK - 1
                    cg = cnt_sb[chunk // CCPG]
                    ctA = cg[:, chunk % CCPG, :, 0:128]
                    ctB = cg[:, chunk % CCPG, :, 128:256]
                    hi = tt[:, c, :, 0:D]
                    lo = tt[:, c, :, D : 2 * D]
                    nc.tensor.matmul(
                        out=psA[:], lhsT=ctA, rhs=hi, start=st, stop=sp_, perf_mode=DR
                    )
                    nc.tensor.matmul(
                        out=psLA[:], lhsT=ctA, rhs=lo, start=st, stop=sp_, perf_mode=DR
                    )
                    nc.tensor.matmul(
                        out=psB[:], lhsT=ctB, rhs=hi, start=st, stop=sp_, perf_mode=DR
                    )
                    nc.tensor.matmul(
                        out=psLB[:], lhsT=ctB, rhs=lo, start=st, stop=sp_, perf_mode=DR
                    )

            # Combine hi + lo/16 into fp16 and push partials for the collective.
            # (walrus allows only one PSUM input per vector op: stage hi first)
            s0a = wp.tile([128, D], fp16)
            nc.vector.tensor_copy(out=s0a[:], in_=psA[:])
            s0 = wp.tile([128, D], fp16)
            nc.vector.scalar_tensor_tensor(
                out=s0[:], in0=psLA[:], scalar=1.0 / LOSCALE, in1=s0a[:],
                op0=mybir.AluOpType.mult, op1=mybir.AluOpType.add,
            )
            s1a = wp.tile([128, D], fp16)
            nc.vector.tensor_copy(out=s1a[:], in_=psB[:])
            s1 = wp.tile([128, D], fp16)
            nc.vector.scalar_tensor_tensor(
                out=s1[:], in0=psLB[:], scalar=1.0 / LOSCALE, in1=s1a[:],
                op0=mybir.AluOpType.mult, op1=mybir.AluOpType.add,
            )
            nc.sync.dma_start(out=cc_in[0:128, :], in_=s0[:])
            nc.scalar.dma_start(out=cc_in[128:256, :], in_=s1[:])

            # Sum partials across cores; rank k keeps docs 32k..32k+31.
            nc.gpsimd.collective_compute(
                "ReduceScatter",
                mybir.AluOpType.add,
                replica_groups=[list(range(NCORES))],
                ins=[cc_in[:]],
                outs=[cc_out[:]],
            )

            ds = wp.tile([BC, D], fp16)
            nc.sync.dma_start(out=ds[:], in_=cc_out[:])

            # avgT[fb] = ds[:, fb]^T @ diag(1/len): transposes to feature-major
            # and applies the segment mean in one tiny matmul per 100-chunk.
            avgT = wp.tile([100, 3 * BC], fp16)
            for fb in range(3):
                pt = pp2.tile([100, BC], f32, tag="mlp")
                nc.tensor.matmul(
                    out=pt[:],
                    lhsT=ds[:, fb * 100 : (fb + 1) * 100],
                    rhs=dinv_sb[:],
                    start=True,
                    stop=True,
                )
                nc.vector.tensor_copy(out=avgT[:, fb * BC : (fb + 1) * BC], in_=pt[:])

            # Layer 1: h1T[j] = relu(W1[:, j-chunk]^T @ avgT + b1), j over 4x128.
            h1 = wp.tile([128, 4 * BC], fp16)
            for j in range(4):
                p1 = pp2.tile([128, BC], f32, tag="mlp")
                for fb in range(3):
                    nc.tensor.matmul(
                        out=p1[:],
                        lhsT=w1_sb[:, (fb * 4 + j) * 128 : (fb * 4 + j + 1) * 128],
                        rhs=avgT[:, fb * BC : (fb + 1) * BC],
                        start=(fb == 0),
                        stop=(fb == 2),
                    )
                nc.scalar.activation(
                    out=h1[:, j * BC : (j + 1) * BC],
                    in_=p1[:],
                    func=relu,
                    bias=b1_sb[:, j : j + 1],
                )

            # Layer 2: h2T[j] = relu(sum_k W2[k-chunk, j-chunk]^T @ h1T[k] + b2).
            h2 = wp.tile([128, 4 * BC], fp16)
            for j in range(4):
                p2 = pp2.tile([128, BC], f32, tag="mlp")
                for k in range(4):
                    nc.tensor.matmul(
                        out=p2[:],
                        lhsT=w2_sb[:, (k * 4 + j) * 128 : (k * 4 + j + 1) * 128],
                        rhs=h1[:, k * BC : (k + 1) * BC],
                        start=(k == 0),
                        stop=(k == 3),
                    )
                nc.scalar.activation(
                    out=h2[:, j * BC : (j + 1) * BC],
                    in_=p2[:],
                    func=relu,
                    bias=b2_sb[:, j : j + 1],
                )

            # Layer 3: out = sum_j h2T[j]^T @ W3[j-chunk] + b3 (bias via K=1 matmul).
            pout = pp2.tile([BC, C], f32, tag="mlp")
            for j in range(4):
                nc.tensor.matmul(
                    out=pout[:],
                    lhsT=h2[:, j * BC : (j + 1) * BC],
                    rhs=w3_sb[:, j * C : (j + 1) * C],
                    start=(j == 0),
                    stop=False,
                )
            nc.tensor.matmul(
                out=pout[:], lhsT=ones_sb[:], rhs=b3_sb[:], start=False, stop=True
            )

            out_sb = wp.tile([BC, C], f32)
            nc.vector.tensor_copy(out=out_sb[:], in_=pout[:])
            nc.sync.dma_start(out=out_d[:], in_=out_sb[:])

    nc.finalize()
    return nc


def _get_nc():
    if "nc" not in _CACHE:
        _CACHE["nc"] = _build_nc()
    return _CACHE["nc"]


def make_in_maps(embeddings, W1, b1, W2, b2, W3, b3, docs, doc_lens):
    """Host-side sharding: fp8 hi/lo table + exact fp8 count shards."""
    import ml_dtypes

    fp8 = ml_dtypes.float8_e4m3
    fp16 = np.float16
    emb = np.asarray(embeddings, np.float32)
    docs = np.asarray(docs, np.int32)
    doc_lens = np.asarray(doc_lens, np.int32)

    # CNT[b, v] = multiplicity of vocab v in doc b.
    ids = (np.arange(B, dtype=np.int64)[:, None] * V + docs.astype(np.int64)).ravel()
    cnt_full = np.bincount(ids, minlength=B * V).reshape(B, V)
    assert cnt_full.max() <= 16, "counts no longer exact in fp8"

    w1 = np.ascontiguousarray(
        np.asarray(W1, np.float32).reshape(3, 100, 4, 128).transpose(1, 0, 2, 3)
        .reshape(100, 12 * 128)
    ).astype(fp16)
    w2 = np.ascontiguousarray(
        np.asarray(W2, np.float32).reshape(4, 128, 4, 128).transpose(1, 0, 2, 3)
        .reshape(128, 16 * 128)
    ).astype(fp16)
    w3 = np.ascontiguousarray(
        np.asarray(W3, np.float32).reshape(4, 128, C).transpose(1, 0, 2).reshape(128, 4 * C)
    ).astype(fp16)
    b1p = np.ascontiguousarray(np.asarray(b1, np.float32).reshape(4, 128).T)
    b2p = np.ascontiguousarray(np.asarray(b2, np.float32).reshape(4, 128).T)
    b3r = np.ascontiguousarray(np.asarray(b3, np.float32).reshape(1, C)).astype(fp16)
    ones = np.ones((1, BC), fp16)

    in_maps = []
    for core in range(NCORES):
        x = np.zeros((VSHP, D), np.float32)
        x[:VSH] = emb[core * VSH : (core + 1) * VSH]
        hi = x.astype(fp8)
        lo = ((x - hi.astype(np.float32)) * LOSCALE).astype(fp8)
        # thl[g, p, c, i, 0:D]=hi row, [.., D:2D]=lo row for vocab row
        # g*(TCPG*256) + c*256 + i*128 + p.
        thl = np.zeros((NTG, 128, TCPG, 2, 2 * D), fp8)
        hi_r = hi.reshape(NTG, TCPG, 2, 128, D).transpose(0, 3, 1, 2, 4)
        lo_r = lo.reshape(NTG, TCPG, 2, 128, D).transpose(0, 3, 1, 2, 4)
        thl[..., :D] = hi_r
        thl[..., D:] = lo_r

        c8 = np.zeros((VSHP, B), fp8)
        c8[:VSH] = cnt_full[:, core * VSH : (core + 1) * VSH].T.astype(fp8)
        # cnt[g, p, c, i, b] = count for vocab row g*(CCPG*256) + c*256 + i*128 + p.
        cnt = np.ascontiguousarray(
            c8.reshape(NCG, CCPG, 2, 128, B).transpose(0, 3, 1, 2, 4)
        )

        dinv = np.zeros((BC, BC), fp16)
        lens = doc_lens[core * BC : (core + 1) * BC].astype(np.float32)
        np.fill_diagonal(dinv, (1.0 / lens).astype(fp16))
        in_maps.append(
            {
                "thl": thl,
                "cnt": cnt,
                "dinv": dinv,
                "w1": w1,
                "w2": w2,
                "w3": w3,
                "b1": b1p,
                "b2": b2p,
                "b3": b3r,
                "ones": ones,
            }
        )
    return in_maps


def kernel(embeddings, W1, b1, W2, b2, W3, b3, docs, doc_lens):
    from concourse.bass_utils import run_bass_kernel_spmd

    nc = _get_nc()
    in_maps = make_in_maps(embeddings, W1, b1, W2, b2, W3, b3, docs, doc_lens)
    res = run_bass_kernel_spmd(nc, in_maps, list(range(NCORES)))
    out = np.concatenate([res.results[i]["out"] for i in range(NCORES)], axis=0)
    return out.astype(np.float32)
